# revision 40
# baseline (speedup 1.0000x reference)
"""Pointer-generator (CopyModule) kernel for Trainium2, 8 NeuronCores.

Math (per batch b, target row t):
    ctx[t,h]   = sum_s attn[t,s] * enc[s,h]
    p[t]       = sigmoid([ctx; dec] @ W_gen + b_gen)
    Z[t]       = sum_v exp(logits[t,v])            (softmax denom, no max-sub:
                                                    logits ~ N(0,1), exp is safe)
    out[t,v]   = ln(p/Z * exp(logits[t,v]) + (1-p) * C[t,v] + 1e-12)
    C[t,v]     = sum_{s: ids[s]==v} attn[t,s]      (scatter-add, nonzero on
                                                    <=512 vocab columns)

Sharding: B*T_tgt = 1024 rows -> 128 rows per core (= SBUF partitions), the
full vocab V on the free axis. Core c handles batch c//2, t-rows (c%2)*128.

The scatter is handled sparsely: the device computes the dense term
ln(p/Z*exp + eps) for all V, plus a small [128, 512] "fix" tensor holding the
corrected values at the <=512 touched vocab columns (C is produced as a
512-wide matmul attn @ D, with D built on-device from the unique-inverse
index vector via iota + is_eq). The host writes fix into the touched columns
of the dense output (pure index-addressed data movement; all FLOPs stay on
device).

Performance notes: the kernel is HBM-bound (~35 MB/core against a ~420 GB/s
per-core cap, measured; read/write direction mix and queue count do not
change the cap). DMA rate tracks per-partition line width, so everything
streams with contiguous >=12 KB lines: logits in 10 chunks of 3210 columns,
all aux tensors pre-laid by the host into one fused [128, 6164] SBUF-layout
block (single transfer, 24.7 KB lines). exp+row-accum overlaps the input
stream; Ln runs in place over the resident exp buffer and streams out.
The only serialization point is the softmax denominator Z.
"""

import os

import numpy as np

import concourse.bass as bass
import concourse.mybir as mybir
import concourse.tile as tile
from concourse import bacc, bass_utils
from concourse.bass import ts

B, T, S, H, V = 4, 256, 512, 1024, 32100
P = 128           # rows per core
NCORES = 8
W = 3210          # vocab chunk width (both streams)
NCH = V // W      # 10 chunks
KU = 512          # padded width of the unique-column (copy) block
F32 = mybir.dt.float32
EPS = 1e-12
AF = mybir.ActivationFunctionType
ALU = mybir.AluOpType

SC = S // P       # 4  s-chunks
HC = H // P       # 8  h-chunks

# offsets into the fused aux block (fp32 elements per partition line)
AT0 = 0                  # attnT   [SC, P]   attn[b, t0+t, sc*128+p]
ENC0 = AT0 + SC * P      # enc     [SC, H]   enc[b, sc*128+p, h]
DEC0 = ENC0 + SC * H     # decT    [HC, P]   dec[b, t0+t, hc*128+p]
W0 = DEC0 + HC * P       # wgen    [2*HC]    wgen[c*128+p, 0]
LGU0 = W0 + 2 * HC       # lgu     [KU]      logits at unique copy columns
INV0 = LGU0 + KU         # inv     [SC]      unique-inverse ids (as fp32)
AUXW = INV0 + SC

_CACHE: dict = {}
LAST_RESULTS = None  # BassKernelResults of the last run (for test harness)


def _ensure_ntff_hook():
    """Register the axon NTFF profiling hook (the agent image's antenv lacks
    the axon_hooks shim module; rebuild it + the ctypes hook ourselves).
    Only needed for KERNEL_TRACE=1 profiling runs; failures are harmless."""
    try:
        import antenv.axon_hooks  # noqa: F401
        return
    except ImportError:
        pass
    try:
        import sys
        import types

        import antenv
        import importlib.util

        spec = importlib.util.find_spec("trn_agent_boot.trn_boot")
        if spec is None:
            sys.path.insert(0, "/root/.axon_site")
        from trn_agent_boot.trn_boot import _ntff_profile_via_ctypes

        mod = types.ModuleType("antenv.axon_hooks")
        mod._hook = _ntff_profile_via_ctypes("/opt/axon/libaxon_pjrt.so")

        def set_axon_ntff_profile_hook(h):
            mod._hook = h

        def get_axon_ntff_profile_hook():
            return mod._hook

        mod.set_axon_ntff_profile_hook = set_axon_ntff_profile_hook
        mod.get_axon_ntff_profile_hook = get_axon_ntff_profile_hook
        sys.modules["antenv.axon_hooks"] = mod
        antenv.axon_hooks = mod
    except Exception as e:  # pragma: no cover
        print(f"NTFF hook setup failed ({e}); tracing disabled")


def _build(bgen: float):
    nc = bacc.Bacc(
        "TRN2", target_bir_lowering=False, debug=False, num_devices=NCORES
    )

    lg = nc.dram_tensor("lg", [P, V], F32, kind="ExternalInput")
    aux = nc.dram_tensor("aux", [P, AUXW], F32, kind="ExternalInput")
    outd = nc.dram_tensor("outd", [P, V], F32, kind="ExternalOutput")
    outf = nc.dram_tensor("outf", [P, KU], F32, kind="ExternalOutput")

    with tile.TileContext(nc) as tc:
        with (
            tc.tile_pool(name="const", bufs=1) as cp,
            tc.tile_pool(name="ps2", bufs=2, space="PSUM") as pp2,
            tc.tile_pool(name="ps1", bufs=1, space="PSUM") as pp1,
        ):
            # input chunks: wide for DMA line efficiency, tapering at the end
            # so the last exp (which gates Z and the whole out phase) is short
            IW = [W] * (NCH - 1) + [2140, 1070]
            expres = cp.tile([P, V], F32)
            zparts = cp.tile([P, len(IW)], F32)

            eps_sb = cp.tile([P, 1], F32)
            nc.vector.memset(eps_sb[:], EPS)
            # dummy Ln before any Exp: the first ACTIVATE decides which
            # activation-table set walrus loads, and the Ln-bearing set also
            # contains Exp — so one startup load covers the whole kernel and
            # no table switch lands on the Z critical path before the first
            # real Ln of the output pass.
            warm = cp.tile([P, 1], F32)
            nc.scalar.activation(out=warm[:], in_=eps_sb[:], func=AF.Ln)

            # one fused aux transfer: contiguous 24.7 KB per-partition lines.
            # It rides the same sync HWDGE ring as the logits stream (a second
            # active ring splits the fabric unevenly and stalls the exp chain),
            # slotted after the third chunk: late enough not to delay the
            # first exps, early enough for the matmul chain to finish p
            # long before Z closes.
            aux_sb = cp.tile([P, AUXW], F32)

            def attnT(sc):
                return aux_sb[:, AT0 + sc * P : AT0 + (sc + 1) * P]

            def enc(sc, hc):
                o = ENC0 + sc * H + hc * P
                return aux_sb[:, o : o + P]

            def decT(c):
                return aux_sb[:, DEC0 + c * P : DEC0 + (c + 1) * P]

            def wcol(c):
                return aux_sb[:, W0 + c : W0 + c + 1]

            lgu_sb = aux_sb[:, LGU0 : LGU0 + KU]

            # pass 1: stream logits directly into the resident buffer and
            # exp in place (chunk slices are independent, so the DMA ring
            # never blocks on a buffer), accumulating partial Z sums
            o = 0
            for i, w in enumerate(IW):
                nc.sync.dma_start(out=expres[:, o : o + w], in_=lg[:, o : o + w])
                nc.scalar.activation(
                    out=expres[:, o : o + w],
                    in_=expres[:, o : o + w],
                    func=AF.Exp,
                    accum_out=zparts[:, i : i + 1],
                )
                o += w
                if i == 2:
                    # matmul-critical aux (attnT/enc/decT/wgen) early; the
                    # copy-path slice (lgu/inv) is deferred past the last
                    # logits chunk so it never delays the Z close
                    nc.sync.dma_start(
                        out=aux_sb[:, :LGU0], in_=aux[:, :LGU0]
                    )
            nc.sync.dma_start(out=aux_sb[:, LGU0:], in_=aux[:, LGU0:])

            # ctxT[h, t] = sum_s enc[s, h] * attn[t, s]
            ctxT_sb = cp.tile([P, HC, P], F32)
            for hc in range(HC):
                pctx = pp2.tile([P, P], F32, tag="ctx")
                for sc in range(SC):
                    nc.tensor.matmul(
                        out=pctx[:],
                        lhsT=enc(sc, hc),
                        rhs=attnT(sc),
                        start=(sc == 0),
                        stop=(sc == SC - 1),
                    )
                nc.vector.tensor_copy(out=ctxT_sb[:, hc, :], in_=pctx[:])

            # p logits: sum_h ctx[t,h] W1[h] + sum_h dec[t,h] W2[h]
            pp_p = pp1.tile([P, 1], F32, tag="p")
            for c in range(HC):
                nc.tensor.matmul(
                    out=pp_p[:],
                    lhsT=ctxT_sb[:, c, :],
                    rhs=wcol(c),
                    start=(c == 0),
                    stop=False,
                )
            for c in range(HC):
                nc.tensor.matmul(
                    out=pp_p[:],
                    lhsT=decT(c),
                    rhs=wcol(HC + c),
                    start=False,
                    stop=(c == HC - 1),
                )

            # sigmoid computed XLA-style (exp-based, no LUT-sigmoid) so that
            # p and 1-p keep full relative precision in both saturation tails
            # and reproduce the reference's fp32 rounding of p near 1.0.
            # xs/ax on the vector engine (exact fp32 add / abs) so the scalar
            # engine's function set is exactly {Exp, Ln}: walrus then loads
            # one combined activation table at startup instead of switching
            # tables on the Z critical path right before the first Ln.
            ones = nc.const_aps.tensor(1.0, (P, 1))
            xs = cp.tile([P, 1], F32)   # x = logit + b_gen
            nc.vector.tensor_scalar_add(out=xs[:], in0=pp_p[:], scalar1=float(bgen))
            ax = cp.tile([P, 1], F32)   # |x| = max(-x, x)
            nc.vector.scalar_tensor_tensor(
                out=ax[:], in0=xs[:], scalar=-1.0, in1=xs[:],
                op0=ALU.mult, op1=ALU.max,
            )
            e1 = cp.tile([P, 1], F32)   # exp(-|x|)
            nc.scalar.activation(out=e1[:], in_=ax[:], func=AF.Exp, scale=-1.0)

            den = cp.tile([P, 1], F32)  # 1 + e
            nc.vector.tensor_scalar_add(out=den[:], in0=e1[:], scalar1=1.0)
            rr = cp.tile([P, 1], F32)   # 1/(1+e)
            nc.vector.reciprocal(out=rr[:], in_=den[:])
            er = cp.tile([P, 1], F32)   # e/(1+e)
            nc.vector.tensor_mul(out=er[:], in0=e1[:], in1=rr[:])
            msk = cp.tile([P, 1], mybir.dt.uint8)  # x >= 0
            nc.vector.tensor_scalar(
                out=msk[:], in0=xs[:], scalar1=0.0, scalar2=None, op0=ALU.is_ge
            )
            p_col = cp.tile([P, 1], F32)
            nc.vector.select(
                out=p_col[:], mask=msk[:], on_true=rr[:], on_false=er[:]
            )
            # 1-p as the reference computes it: exact fp32 subtraction from
            # the already-rounded p (p=1/fp32(1+e) carries the rounding, so
            # 1-p inherits the reference's ladder/flush behavior bit-for-bit)
            omp = cp.tile([P, 1], F32)  # 1 - p
            nc.vector.scalar_tensor_tensor(
                out=omp[:], in0=p_col[:], scalar=-1.0, in1=ones,
                op0=ALU.mult, op1=ALU.add,
            )

            # Z -> 1/Z -> p/Z
            zsum = cp.tile([P, 1], F32)
            nc.vector.tensor_reduce(
                out=zsum[:], in_=zparts[:], axis=mybir.AxisListType.X, op=ALU.add
            )
            rz = cp.tile([P, 1], F32)
            nc.vector.reciprocal(out=rz[:], in_=zsum[:])
            pz = cp.tile([P, 1], F32)
            nc.vector.tensor_mul(out=pz[:], in0=p_col[:], in1=rz[:])

            # everything below the vector pz chain runs post-Z inside the
            # out-stream's DMA slack: the dmat build (reads the deferred aux
            # slice), the C matmul, and the copy-path fix.
            # D[s, k] = 1 iff inv[s] == k, built on-device (f32 exact)
            iota_sb = cp.tile([P, KU], F32)
            nc.gpsimd.iota(
                iota_sb[:], pattern=[[1, KU]], channel_multiplier=0,
                allow_small_or_imprecise_dtypes=True,
            )
            dmat_sb = cp.tile([P, SC, KU], F32)
            for sc in range(SC):
                nc.vector.tensor_scalar(
                    out=dmat_sb[:, sc, :], in0=iota_sb[:],
                    scalar1=aux_sb[:, INV0 + sc : INV0 + sc + 1],
                    scalar2=None, op0=ALU.is_equal,
                )
            # copy-path matmul C = attn @ D
            pC = pp1.tile([P, KU], F32, tag="C")
            for sc in range(SC):
                nc.tensor.matmul(
                    out=pC[:],
                    lhsT=attnT(sc),
                    rhs=dmat_sb[:, sc, :],
                    start=(sc == 0),
                    stop=(sc == SC - 1),
                )

            # pass 2: dense ln(pz*exp + eps) computed in place over the
            # resident exp buffer, streamed straight out. The leading chunks
            # are narrow so the out stream starts as soon after Z as possible
            # (the stream end is DMA-bound, the start gated by the Exp->Ln
            # activation-table switch plus the first Ln). The copy-path
            # fix = ln(pz*exp(lgu) + (1-p)*C + eps) is slotted in mid-stream
            # on the otherwise-idle gpsimd ring once its inputs are safely
            # ready; the out-DMA slack absorbs its scalar time.
            OW = [535, 535, 1070, 1070] + [3210] * 9
            o = 0
            for i, w in enumerate(OW):
                nc.scalar.activation(
                    out=expres[:, o : o + w],
                    in_=expres[:, o : o + w],
                    func=AF.Ln,
                    scale=pz[:, :1],
                    bias=eps_sb[:, :1],
                )
                nc.sync.dma_start(out=outd[:, o : o + w], in_=expres[:, o : o + w])
                o += w

            # copy-path fix, after the whole dense stream: the two extra
            # activation-table switches (Ln->Exp for exp(lgu), back to Ln)
            # and the fix math all hide under the out stream's DMA backlog
            # drain; the small fix DMA rides the idle gpsimd ring.
            cs = cp.tile([P, KU], F32)
            nc.vector.tensor_scalar(
                out=cs[:], in0=pC[:], scalar1=omp[:, :1],
                scalar2=None, op0=ALU.mult,
            )
            gexp = cp.tile([P, KU], F32)
            nc.scalar.activation(out=gexp[:], in_=lgu_sb, func=AF.Exp)
            fx = cp.tile([P, KU], F32)
            nc.vector.scalar_tensor_tensor(
                out=fx[:],
                in0=gexp[:],
                scalar=pz[:, :1],
                in1=cs[:],
                op0=ALU.mult,
                op1=ALU.add,
            )
            nc.scalar.activation(
                out=fx[:], in_=fx[:], func=AF.Ln, bias=eps_sb[:, :1]
            )
            nc.gpsimd.dma_start(out=outf[:], in_=fx[:])

    nc.compile()
    return nc


def _host_prep(core: int, logits, attn, enc, dec, wgen, ids):
    """Build one core's pre-laid-out input map (+ its unique-id vector)."""
    b, half = divmod(core, T // P)
    t0 = half * P
    u, inv = np.unique(ids[b], return_inverse=True)

    aux = np.empty((P, AUXW), np.float32)
    # attnT[p, sc*P + t] = attn[b, t0+t, sc*128+p]
    aux[:, AT0:ENC0] = (
        attn[b, t0 : t0 + P].T.reshape(SC, P, P).transpose(1, 0, 2).reshape(P, SC * P)
    )
    # enc[p, sc*H + h] = enc[b, sc*128+p, h]
    aux[:, ENC0:DEC0] = (
        enc[b].reshape(SC, P, H).transpose(1, 0, 2).reshape(P, SC * H)
    )
    # decT[p, c*P + t] = dec[b, t0+t, c*128+p]
    aux[:, DEC0:W0] = (
        dec[b, t0 : t0 + P].T.reshape(HC, P, P).transpose(1, 0, 2).reshape(P, HC * P)
    )
    # wgen[p, c] = W_gen[c*128+p, 0]
    aux[:, W0:LGU0] = wgen.reshape(2 * HC, P).T
    aux[:, LGU0:INV0] = 0.0
    aux[:, LGU0 : LGU0 + len(u)] = logits[b, t0 : t0 + P][:, u]
    # inv[p, sc] = unique-inverse id of source position sc*128+p (exact in f32)
    aux[:, INV0:AUXW] = inv.astype(np.float32).reshape(SC, P).T

    return {
        "lg": np.ascontiguousarray(logits[b, t0 : t0 + P]),
        "aux": np.ascontiguousarray(aux),
    }, u


def kernel(**inputs) -> np.ndarray:
    global LAST_RESULTS
    dec = np.asarray(inputs["decoder_hidden_states"], dtype=np.float32)
    attn = np.asarray(inputs["cross_attention_weights"], dtype=np.float32)
    enc = np.asarray(inputs["encoder_hidden_states"], dtype=np.float32)
    logits = np.asarray(inputs["vocab_logits"], dtype=np.float32)
    wgen = np.asarray(inputs["W_gen"], dtype=np.float32).reshape(2 * H, 1)
    bgen = float(np.asarray(inputs["b_gen"]).reshape(-1)[0])
    ids = np.asarray(inputs["source_ids"]).astype(np.int64)

    key = bgen
    nc = _CACHE.get(key)
    if nc is None:
        nc = _build(bgen)
        _CACHE[key] = nc

    uniqs = []
    in_maps = []
    for core in range(NCORES):
        m, u = _host_prep(core, logits, attn, enc, dec, wgen, ids)
        in_maps.append(m)
        uniqs.append(u)

    trace = bool(os.environ.get("KERNEL_TRACE"))
    if trace:
        _ensure_ntff_hook()
    res = bass_utils.run_bass_kernel_spmd(
        nc,
        in_maps,
        core_ids=list(range(NCORES)),
        trace=trace,
    )
    LAST_RESULTS = res

    out = np.empty((B, T, V), np.float32)
    for core in range(NCORES):
        b, half = divmod(core, T // P)
        t0 = half * P
        r = res.results[core]
        out[b, t0 : t0 + P] = r["outd"]
        u = uniqs[core]
        out[b, t0 : t0 + P, :][:, u] = r["outf"][:, : len(u)]
    return out


# revision 41
# speedup vs baseline: 1.0178x; 1.0178x over previous
"""Pointer-generator (CopyModule) kernel for Trainium2, 8 NeuronCores.

Math (per batch b, target row t):
    ctx[t,h]   = sum_s attn[t,s] * enc[s,h]
    p[t]       = sigmoid([ctx; dec] @ W_gen + b_gen)
    Z[t]       = sum_v exp(logits[t,v])            (softmax denom, no max-sub:
                                                    logits ~ N(0,1), exp is safe)
    out[t,v]   = ln(p/Z * exp(logits[t,v]) + (1-p) * C[t,v] + 1e-12)
    C[t,v]     = sum_{s: ids[s]==v} attn[t,s]      (scatter-add, nonzero on
                                                    <=512 vocab columns)

Sharding: B*T_tgt = 1024 rows -> 128 rows per core (= SBUF partitions), the
full vocab V on the free axis. Core c handles batch c//2, t-rows (c%2)*128.

The scatter is handled sparsely: the device computes the dense term
ln(p/Z*exp + eps) for all V, plus a small [128, 512] "fix" tensor holding the
corrected values at the <=512 touched vocab columns (C is produced as a
512-wide matmul attn @ D, with D built on-device from the unique-inverse
index vector via iota + is_eq). The host writes fix into the touched columns
of the dense output (pure index-addressed data movement; all FLOPs stay on
device).

Performance notes: the kernel is HBM-bound (~35 MB/core against a ~420 GB/s
per-core cap, measured; read/write direction mix and queue count do not
change the cap). DMA rate tracks per-partition line width, so everything
streams with contiguous >=12 KB lines: logits in 10 chunks of 3210 columns,
all aux tensors pre-laid by the host into one fused [128, 6164] SBUF-layout
block (single transfer, 24.7 KB lines). exp+row-accum overlaps the input
stream; Ln runs in place over the resident exp buffer and streams out.
The only serialization point is the softmax denominator Z.
"""

import os

import numpy as np

import concourse.bass as bass
import concourse.mybir as mybir
import concourse.tile as tile
from concourse import bacc, bass_utils
from concourse.bass import ts

B, T, S, H, V = 4, 256, 512, 1024, 32100
P = 128           # rows per core
NCORES = 8
W = 3210          # vocab chunk width (both streams)
NCH = V // W      # 10 chunks
KU = 512          # padded width of the unique-column (copy) block
F32 = mybir.dt.float32
EPS = 1e-12
AF = mybir.ActivationFunctionType
ALU = mybir.AluOpType

SC = S // P       # 4  s-chunks
HC = H // P       # 8  h-chunks

# offsets into the fused aux block (fp32 elements per partition line)
AT0 = 0                  # attnT   [SC, P]   attn[b, t0+t, sc*128+p]
ENC0 = AT0 + SC * P      # enc     [SC, H]   enc[b, sc*128+p, h]
DEC0 = ENC0 + SC * H     # decT    [HC, P]   dec[b, t0+t, hc*128+p]
W0 = DEC0 + HC * P       # wgen    [2*HC]    wgen[c*128+p, 0]
LGU0 = W0 + 2 * HC       # lgu     [KU]      logits at unique copy columns
INV0 = LGU0 + KU         # inv     [SC]      unique-inverse ids (as fp32)
AUXW = INV0 + SC

_CACHE: dict = {}
LAST_RESULTS = None  # BassKernelResults of the last run (for test harness)


def _ensure_ntff_hook():
    """Register the axon NTFF profiling hook (the agent image's antenv lacks
    the axon_hooks shim module; rebuild it + the ctypes hook ourselves).
    Only needed for KERNEL_TRACE=1 profiling runs; failures are harmless."""
    try:
        import antenv.axon_hooks  # noqa: F401
        return
    except ImportError:
        pass
    try:
        import sys
        import types

        import antenv
        import importlib.util

        spec = importlib.util.find_spec("trn_agent_boot.trn_boot")
        if spec is None:
            sys.path.insert(0, "/root/.axon_site")
        from trn_agent_boot.trn_boot import _ntff_profile_via_ctypes

        mod = types.ModuleType("antenv.axon_hooks")
        mod._hook = _ntff_profile_via_ctypes("/opt/axon/libaxon_pjrt.so")

        def set_axon_ntff_profile_hook(h):
            mod._hook = h

        def get_axon_ntff_profile_hook():
            return mod._hook

        mod.set_axon_ntff_profile_hook = set_axon_ntff_profile_hook
        mod.get_axon_ntff_profile_hook = get_axon_ntff_profile_hook
        sys.modules["antenv.axon_hooks"] = mod
        antenv.axon_hooks = mod
    except Exception as e:  # pragma: no cover
        print(f"NTFF hook setup failed ({e}); tracing disabled")


def _build(bgen: float):
    nc = bacc.Bacc(
        "TRN2", target_bir_lowering=False, debug=False, num_devices=NCORES
    )

    lg = nc.dram_tensor("lg", [P, V], F32, kind="ExternalInput")
    aux = nc.dram_tensor("aux", [P, AUXW], F32, kind="ExternalInput")
    outd = nc.dram_tensor("outd", [P, V], F32, kind="ExternalOutput")
    outf = nc.dram_tensor("outf", [P, KU], F32, kind="ExternalOutput")

    with tile.TileContext(nc) as tc:
        with (
            tc.tile_pool(name="const", bufs=1) as cp,
            tc.tile_pool(name="ps2", bufs=2, space="PSUM") as pp2,
            tc.tile_pool(name="ps1", bufs=1, space="PSUM") as pp1,
        ):
            # input chunks: wide for DMA line efficiency, tapering at the end
            # so the last exp (which gates Z and the whole out phase) is short
            IW = [W] * (NCH - 1) + [2140, 1070]
            expres = cp.tile([P, V], F32)
            zparts = cp.tile([P, len(IW)], F32)

            eps_sb = cp.tile([P, 1], F32)
            nc.vector.memset(eps_sb[:], EPS)

            # one fused aux transfer: contiguous 24.7 KB per-partition lines.
            # It rides the same sync HWDGE ring as the logits stream (a second
            # active ring splits the fabric unevenly and stalls the exp chain),
            # slotted after the third chunk: late enough not to delay the
            # first exps, early enough for the matmul chain to finish p
            # long before Z closes.
            aux_sb = cp.tile([P, AUXW], F32)

            def attnT(sc):
                return aux_sb[:, AT0 + sc * P : AT0 + (sc + 1) * P]

            def enc(sc, hc):
                o = ENC0 + sc * H + hc * P
                return aux_sb[:, o : o + P]

            def decT(c):
                return aux_sb[:, DEC0 + c * P : DEC0 + (c + 1) * P]

            def wcol(c):
                return aux_sb[:, W0 + c : W0 + c + 1]

            lgu_sb = aux_sb[:, LGU0 : LGU0 + KU]

            # pass 1: stream logits directly into the resident buffer and
            # exp in place (chunk slices are independent, so the DMA ring
            # never blocks on a buffer), accumulating partial Z sums
            o = 0
            for i, w in enumerate(IW):
                nc.sync.dma_start(out=expres[:, o : o + w], in_=lg[:, o : o + w])
                nc.scalar.activation(
                    out=expres[:, o : o + w],
                    in_=expres[:, o : o + w],
                    func=AF.Exp,
                    accum_out=zparts[:, i : i + 1],
                )
                o += w
                if i == 2:
                    # matmul-critical aux (attnT/enc/decT/wgen) early; the
                    # copy-path slice (lgu/inv) is deferred past the last
                    # logits chunk so it never delays the Z close
                    nc.sync.dma_start(
                        out=aux_sb[:, :LGU0], in_=aux[:, :LGU0]
                    )
            nc.sync.dma_start(out=aux_sb[:, LGU0:], in_=aux[:, LGU0:])

            # ctxT[h, t] = sum_s enc[s, h] * attn[t, s]
            ctxT_sb = cp.tile([P, HC, P], F32)
            for hc in range(HC):
                pctx = pp2.tile([P, P], F32, tag="ctx")
                for sc in range(SC):
                    nc.tensor.matmul(
                        out=pctx[:],
                        lhsT=enc(sc, hc),
                        rhs=attnT(sc),
                        start=(sc == 0),
                        stop=(sc == SC - 1),
                    )
                nc.vector.tensor_copy(out=ctxT_sb[:, hc, :], in_=pctx[:])

            # p logits: sum_h ctx[t,h] W1[h] + sum_h dec[t,h] W2[h]
            pp_p = pp1.tile([P, 1], F32, tag="p")
            for c in range(HC):
                nc.tensor.matmul(
                    out=pp_p[:],
                    lhsT=ctxT_sb[:, c, :],
                    rhs=wcol(c),
                    start=(c == 0),
                    stop=False,
                )
            for c in range(HC):
                nc.tensor.matmul(
                    out=pp_p[:],
                    lhsT=decT(c),
                    rhs=wcol(HC + c),
                    start=False,
                    stop=(c == HC - 1),
                )

            # sigmoid computed XLA-style (exp-based, no LUT-sigmoid) so that
            # p and 1-p keep full relative precision in both saturation tails
            # and reproduce the reference's fp32 rounding of p near 1.0.
            # xs/ax on the vector engine (exact fp32 add / abs) so the scalar
            # engine's function set is exactly {Exp, Ln}: walrus then loads
            # one combined activation table at startup instead of switching
            # tables on the Z critical path right before the first Ln.
            ones = nc.const_aps.tensor(1.0, (P, 1))
            xs = cp.tile([P, 1], F32)   # x = logit + b_gen
            nc.vector.tensor_scalar_add(out=xs[:], in0=pp_p[:], scalar1=float(bgen))
            ax = cp.tile([P, 1], F32)   # |x| = max(-x, x)
            nc.vector.scalar_tensor_tensor(
                out=ax[:], in0=xs[:], scalar=-1.0, in1=xs[:],
                op0=ALU.mult, op1=ALU.max,
            )
            e1 = cp.tile([P, 1], F32)   # exp(-|x|)
            nc.scalar.activation(out=e1[:], in_=ax[:], func=AF.Exp, scale=-1.0)

            den = cp.tile([P, 1], F32)  # 1 + e
            nc.vector.tensor_scalar_add(out=den[:], in0=e1[:], scalar1=1.0)
            rr = cp.tile([P, 1], F32)   # 1/(1+e)
            nc.vector.reciprocal(out=rr[:], in_=den[:])
            er = cp.tile([P, 1], F32)   # e/(1+e)
            nc.vector.tensor_mul(out=er[:], in0=e1[:], in1=rr[:])
            msk = cp.tile([P, 1], mybir.dt.uint8)  # x >= 0
            nc.vector.tensor_scalar(
                out=msk[:], in0=xs[:], scalar1=0.0, scalar2=None, op0=ALU.is_ge
            )
            p_col = cp.tile([P, 1], F32)
            nc.vector.select(
                out=p_col[:], mask=msk[:], on_true=rr[:], on_false=er[:]
            )
            # 1-p as the reference computes it: exact fp32 subtraction from
            # the already-rounded p (p=1/fp32(1+e) carries the rounding, so
            # 1-p inherits the reference's ladder/flush behavior bit-for-bit)
            omp = cp.tile([P, 1], F32)  # 1 - p
            nc.vector.scalar_tensor_tensor(
                out=omp[:], in0=p_col[:], scalar=-1.0, in1=ones,
                op0=ALU.mult, op1=ALU.add,
            )

            # Z -> 1/Z -> p/Z
            zsum = cp.tile([P, 1], F32)
            nc.vector.tensor_reduce(
                out=zsum[:], in_=zparts[:], axis=mybir.AxisListType.X, op=ALU.add
            )
            rz = cp.tile([P, 1], F32)
            nc.vector.reciprocal(out=rz[:], in_=zsum[:])
            pz = cp.tile([P, 1], F32)
            nc.vector.tensor_mul(out=pz[:], in0=p_col[:], in1=rz[:])

            # everything below the vector pz chain runs post-Z inside the
            # out-stream's DMA slack: the dmat build (reads the deferred aux
            # slice), the C matmul, and the copy-path fix.
            # D[s, k] = 1 iff inv[s] == k, built on-device (f32 exact)
            iota_sb = cp.tile([P, KU], F32)
            nc.gpsimd.iota(
                iota_sb[:], pattern=[[1, KU]], channel_multiplier=0,
                allow_small_or_imprecise_dtypes=True,
            )
            dmat_sb = cp.tile([P, SC, KU], F32)
            for sc in range(SC):
                nc.vector.tensor_scalar(
                    out=dmat_sb[:, sc, :], in0=iota_sb[:],
                    scalar1=aux_sb[:, INV0 + sc : INV0 + sc + 1],
                    scalar2=None, op0=ALU.is_equal,
                )
            # copy-path matmul C = attn @ D
            pC = pp1.tile([P, KU], F32, tag="C")
            for sc in range(SC):
                nc.tensor.matmul(
                    out=pC[:],
                    lhsT=attnT(sc),
                    rhs=dmat_sb[:, sc, :],
                    start=(sc == 0),
                    stop=(sc == SC - 1),
                )

            # pass 2: dense ln(pz*exp + eps) computed in place over the
            # resident exp buffer, streamed straight out. The leading chunks
            # are narrow so the out stream starts as soon after Z as possible
            # (the stream end is DMA-bound, the start gated by the Exp->Ln
            # activation-table switch plus the first Ln). The copy-path
            # fix = ln(pz*exp(lgu) + (1-p)*C + eps) is slotted in mid-stream
            # on the otherwise-idle gpsimd ring once its inputs are safely
            # ready; the out-DMA slack absorbs its scalar time.
            OW = [535, 535, 1070, 1070] + [3210] * 9
            o = 0
            for i, w in enumerate(OW):
                nc.scalar.activation(
                    out=expres[:, o : o + w],
                    in_=expres[:, o : o + w],
                    func=AF.Ln,
                    scale=pz[:, :1],
                    bias=eps_sb[:, :1],
                )
                nc.sync.dma_start(out=outd[:, o : o + w], in_=expres[:, o : o + w])
                o += w

            # copy-path fix, after the whole dense stream: the two extra
            # activation-table switches (Ln->Exp for exp(lgu), back to Ln)
            # and the fix math all hide under the out stream's DMA backlog
            # drain; the small fix DMA rides the idle gpsimd ring.
            cs = cp.tile([P, KU], F32)
            nc.vector.tensor_scalar(
                out=cs[:], in0=pC[:], scalar1=omp[:, :1],
                scalar2=None, op0=ALU.mult,
            )
            gexp = cp.tile([P, KU], F32)
            nc.scalar.activation(out=gexp[:], in_=lgu_sb, func=AF.Exp)
            fx = cp.tile([P, KU], F32)
            nc.vector.scalar_tensor_tensor(
                out=fx[:],
                in0=gexp[:],
                scalar=pz[:, :1],
                in1=cs[:],
                op0=ALU.mult,
                op1=ALU.add,
            )
            nc.scalar.activation(
                out=fx[:], in_=fx[:], func=AF.Ln, bias=eps_sb[:, :1]
            )
            nc.gpsimd.dma_start(out=outf[:], in_=fx[:])

    nc.compile()
    return nc


def _host_prep(core: int, logits, attn, enc, dec, wgen, ids):
    """Build one core's pre-laid-out input map (+ its unique-id vector)."""
    b, half = divmod(core, T // P)
    t0 = half * P
    u, inv = np.unique(ids[b], return_inverse=True)

    aux = np.empty((P, AUXW), np.float32)
    # attnT[p, sc*P + t] = attn[b, t0+t, sc*128+p]
    aux[:, AT0:ENC0] = (
        attn[b, t0 : t0 + P].T.reshape(SC, P, P).transpose(1, 0, 2).reshape(P, SC * P)
    )
    # enc[p, sc*H + h] = enc[b, sc*128+p, h]
    aux[:, ENC0:DEC0] = (
        enc[b].reshape(SC, P, H).transpose(1, 0, 2).reshape(P, SC * H)
    )
    # decT[p, c*P + t] = dec[b, t0+t, c*128+p]
    aux[:, DEC0:W0] = (
        dec[b, t0 : t0 + P].T.reshape(HC, P, P).transpose(1, 0, 2).reshape(P, HC * P)
    )
    # wgen[p, c] = W_gen[c*128+p, 0]
    aux[:, W0:LGU0] = wgen.reshape(2 * HC, P).T
    aux[:, LGU0:INV0] = 0.0
    aux[:, LGU0 : LGU0 + len(u)] = logits[b, t0 : t0 + P][:, u]
    # inv[p, sc] = unique-inverse id of source position sc*128+p (exact in f32)
    aux[:, INV0:AUXW] = inv.astype(np.float32).reshape(SC, P).T

    return {
        "lg": np.ascontiguousarray(logits[b, t0 : t0 + P]),
        "aux": np.ascontiguousarray(aux),
    }, u


def kernel(**inputs) -> np.ndarray:
    global LAST_RESULTS
    dec = np.asarray(inputs["decoder_hidden_states"], dtype=np.float32)
    attn = np.asarray(inputs["cross_attention_weights"], dtype=np.float32)
    enc = np.asarray(inputs["encoder_hidden_states"], dtype=np.float32)
    logits = np.asarray(inputs["vocab_logits"], dtype=np.float32)
    wgen = np.asarray(inputs["W_gen"], dtype=np.float32).reshape(2 * H, 1)
    bgen = float(np.asarray(inputs["b_gen"]).reshape(-1)[0])
    ids = np.asarray(inputs["source_ids"]).astype(np.int64)

    key = bgen
    nc = _CACHE.get(key)
    if nc is None:
        nc = _build(bgen)
        _CACHE[key] = nc

    uniqs = []
    in_maps = []
    for core in range(NCORES):
        m, u = _host_prep(core, logits, attn, enc, dec, wgen, ids)
        in_maps.append(m)
        uniqs.append(u)

    trace = bool(os.environ.get("KERNEL_TRACE"))
    if trace:
        _ensure_ntff_hook()
    res = bass_utils.run_bass_kernel_spmd(
        nc,
        in_maps,
        core_ids=list(range(NCORES)),
        trace=trace,
    )
    LAST_RESULTS = res

    out = np.empty((B, T, V), np.float32)
    for core in range(NCORES):
        b, half = divmod(core, T // P)
        t0 = half * P
        r = res.results[core]
        out[b, t0 : t0 + P] = r["outd"]
        u = uniqs[core]
        out[b, t0 : t0 + P, :][:, u] = r["outf"][:, : len(u)]
    return out


# revision 42
# speedup vs baseline: 1.1397x; 1.1198x over previous
"""Pointer-generator (CopyModule) kernel for Trainium2, 8 NeuronCores.

Math (per batch b, target row t):
    ctx[t,h]   = sum_s attn[t,s] * enc[s,h]
    p[t]       = sigmoid([ctx; dec] @ W_gen + b_gen)
    Z[t]       = sum_v exp(logits[t,v])            (softmax denom, no max-sub:
                                                    logits ~ N(0,1), exp is safe)
    out[t,v]   = ln(p/Z * exp(logits[t,v]) + (1-p) * C[t,v] + 1e-12)
    C[t,v]     = sum_{s: ids[s]==v} attn[t,s]      (scatter-add, nonzero on
                                                    <=512 vocab columns)

Sharding: B*T_tgt = 1024 rows -> 128 rows per core (= SBUF partitions), the
full vocab V on the free axis. Core c handles batch c//2, t-rows (c%2)*128.

The scatter is handled sparsely: the device computes the dense term
ln(p/Z*exp + eps) for all V, plus a small [128, 512] "fix" tensor holding the
corrected values at the <=512 touched vocab columns (C is produced as a
512-wide matmul attn @ D, with D built on-device from the unique-inverse
index vector via iota + is_eq). The host writes fix into the touched columns
of the dense output (pure index-addressed data movement; all FLOPs stay on
device).

Performance notes: the kernel is HBM-bound (~35 MB/core against a ~420 GB/s
per-core cap, measured; read/write direction mix and queue count do not
change the cap). DMA rate tracks per-partition line width, so everything
streams with contiguous >=12 KB lines: logits in 10 chunks of 3210 columns,
all aux tensors pre-laid by the host into one fused [128, 6164] SBUF-layout
block (single transfer, 24.7 KB lines). exp+row-accum overlaps the input
stream; Ln runs in place over the resident exp buffer and streams out.
The only serialization point is the softmax denominator Z.
"""

import os

import numpy as np

import concourse.bass as bass
import concourse.mybir as mybir
import concourse.tile as tile
from concourse import bacc, bass_utils
from concourse.bass import ts

B, T, S, H, V = 4, 256, 512, 1024, 32100
P = 128           # rows per core
NCORES = 8
W = 3210          # vocab chunk width (both streams)
NCH = V // W      # 10 chunks
KU = 512          # padded width of the unique-column (copy) block
F32 = mybir.dt.float32
EPS = 1e-12
AF = mybir.ActivationFunctionType
ALU = mybir.AluOpType

SC = S // P       # 4  s-chunks
HC = H // P       # 8  h-chunks

# offsets into the fused aux block (fp32 elements per partition line)
AT0 = 0                  # attnT   [SC, P]   attn[b, t0+t, sc*128+p]
ENC0 = AT0 + SC * P      # enc     [SC, H]   enc[b, sc*128+p, h]
DEC0 = ENC0 + SC * H     # decT    [HC, P]   dec[b, t0+t, hc*128+p]
W0 = DEC0 + HC * P       # wgen    [2*HC]    wgen[c*128+p, 0]
LGU0 = W0 + 2 * HC       # lgu     [KU]      logits at unique copy columns
INV0 = LGU0 + KU         # inv     [SC]      unique-inverse ids (as fp32)
AUXW = INV0 + SC

_CACHE: dict = {}
LAST_RESULTS = None  # BassKernelResults of the last run (for test harness)


def _ensure_ntff_hook():
    """Register the axon NTFF profiling hook (the agent image's antenv lacks
    the axon_hooks shim module; rebuild it + the ctypes hook ourselves).
    Only needed for KERNEL_TRACE=1 profiling runs; failures are harmless."""
    try:
        import antenv.axon_hooks  # noqa: F401
        return
    except ImportError:
        pass
    try:
        import sys
        import types

        import antenv
        import importlib.util

        spec = importlib.util.find_spec("trn_agent_boot.trn_boot")
        if spec is None:
            sys.path.insert(0, "/root/.axon_site")
        from trn_agent_boot.trn_boot import _ntff_profile_via_ctypes

        mod = types.ModuleType("antenv.axon_hooks")
        mod._hook = _ntff_profile_via_ctypes("/opt/axon/libaxon_pjrt.so")

        def set_axon_ntff_profile_hook(h):
            mod._hook = h

        def get_axon_ntff_profile_hook():
            return mod._hook

        mod.set_axon_ntff_profile_hook = set_axon_ntff_profile_hook
        mod.get_axon_ntff_profile_hook = get_axon_ntff_profile_hook
        sys.modules["antenv.axon_hooks"] = mod
        antenv.axon_hooks = mod
    except Exception as e:  # pragma: no cover
        print(f"NTFF hook setup failed ({e}); tracing disabled")


def _build(bgen: float):
    nc = bacc.Bacc(
        "TRN2", target_bir_lowering=False, debug=False, num_devices=NCORES
    )

    lg = nc.dram_tensor("lg", [P, V], F32, kind="ExternalInput")
    aux = nc.dram_tensor("aux", [P, AUXW], F32, kind="ExternalInput")
    outd = nc.dram_tensor("outd", [P, V], F32, kind="ExternalOutput")
    outf = nc.dram_tensor("outf", [P, KU], F32, kind="ExternalOutput")

    with tile.TileContext(nc) as tc:
        with (
            tc.tile_pool(name="const", bufs=1) as cp,
            tc.tile_pool(name="ps2", bufs=2, space="PSUM") as pp2,
            tc.tile_pool(name="ps1", bufs=1, space="PSUM") as pp1,
        ):
            # input chunks: wide for DMA line efficiency, tapering at the end
            # so the last exp (which gates Z and the whole out phase) is short
            IW = [W] * (NCH - 1) + [2140, 1070]
            expres = cp.tile([P, V], F32)
            zparts = cp.tile([P, len(IW)], F32)

            eps_sb = cp.tile([P, 1], F32)
            nc.vector.memset(eps_sb[:], EPS)

            # one fused aux transfer: contiguous 24.7 KB per-partition lines.
            # It rides the same sync HWDGE ring as the logits stream (a second
            # active ring splits the fabric unevenly and stalls the exp chain),
            # slotted after the third chunk: late enough not to delay the
            # first exps, early enough for the matmul chain to finish p
            # long before Z closes.
            aux_sb = cp.tile([P, AUXW], F32)

            def attnT(sc):
                return aux_sb[:, AT0 + sc * P : AT0 + (sc + 1) * P]

            def enc(sc, hc):
                o = ENC0 + sc * H + hc * P
                return aux_sb[:, o : o + P]

            def decT(c):
                return aux_sb[:, DEC0 + c * P : DEC0 + (c + 1) * P]

            def wcol(c):
                return aux_sb[:, W0 + c : W0 + c + 1]

            lgu_sb = aux_sb[:, LGU0 : LGU0 + KU]

            # pass 1: stream logits directly into the resident buffer and
            # exp in place (chunk slices are independent, so the DMA ring
            # never blocks on a buffer), accumulating partial Z sums
            o = 0
            for i, w in enumerate(IW):
                nc.sync.dma_start(out=expres[:, o : o + w], in_=lg[:, o : o + w])
                nc.scalar.activation(
                    out=expres[:, o : o + w],
                    in_=expres[:, o : o + w],
                    func=AF.Exp,
                    accum_out=zparts[:, i : i + 1],
                )
                o += w
                if i == 2:
                    # matmul-critical aux (attnT/enc/decT/wgen) early; the
                    # copy-path slice (lgu/inv) is deferred past the last
                    # logits chunk so it never delays the Z close
                    nc.sync.dma_start(
                        out=aux_sb[:, :LGU0], in_=aux[:, :LGU0]
                    )
            nc.sync.dma_start(out=aux_sb[:, LGU0:], in_=aux[:, LGU0:])

            # ctxT[h, t] = sum_s enc[s, h] * attn[t, s]
            ctxT_sb = cp.tile([P, HC, P], F32)
            for hc in range(HC):
                pctx = pp2.tile([P, P], F32, tag="ctx")
                for sc in range(SC):
                    nc.tensor.matmul(
                        out=pctx[:],
                        lhsT=enc(sc, hc),
                        rhs=attnT(sc),
                        start=(sc == 0),
                        stop=(sc == SC - 1),
                    )
                nc.vector.tensor_copy(out=ctxT_sb[:, hc, :], in_=pctx[:])

            # p logits: sum_h ctx[t,h] W1[h] + sum_h dec[t,h] W2[h]
            pp_p = pp1.tile([P, 1], F32, tag="p")
            for c in range(HC):
                nc.tensor.matmul(
                    out=pp_p[:],
                    lhsT=ctxT_sb[:, c, :],
                    rhs=wcol(c),
                    start=(c == 0),
                    stop=False,
                )
            for c in range(HC):
                nc.tensor.matmul(
                    out=pp_p[:],
                    lhsT=decT(c),
                    rhs=wcol(HC + c),
                    start=False,
                    stop=(c == HC - 1),
                )

            # sigmoid computed XLA-style (exp-based, no LUT-sigmoid) so that
            # p and 1-p keep full relative precision in both saturation tails
            # and reproduce the reference's fp32 rounding of p near 1.0.
            # xs/ax on the vector engine (exact fp32 add / abs) so the scalar
            # engine's function set is exactly {Exp, Ln}: walrus then loads
            # one combined activation table at startup instead of switching
            # tables on the Z critical path right before the first Ln.
            ones = nc.const_aps.tensor(1.0, (P, 1))
            xs = cp.tile([P, 1], F32)   # x = logit + b_gen
            nc.vector.tensor_scalar_add(out=xs[:], in0=pp_p[:], scalar1=float(bgen))
            ax = cp.tile([P, 1], F32)   # |x| = max(-x, x)
            nc.vector.scalar_tensor_tensor(
                out=ax[:], in0=xs[:], scalar=-1.0, in1=xs[:],
                op0=ALU.mult, op1=ALU.max,
            )
            e1 = cp.tile([P, 1], F32)   # exp(-|x|)
            nc.scalar.activation(out=e1[:], in_=ax[:], func=AF.Exp, scale=-1.0)

            den = cp.tile([P, 1], F32)  # 1 + e
            nc.vector.tensor_scalar_add(out=den[:], in0=e1[:], scalar1=1.0)
            rr = cp.tile([P, 1], F32)   # 1/(1+e)
            nc.vector.reciprocal(out=rr[:], in_=den[:])
            er = cp.tile([P, 1], F32)   # e/(1+e)
            nc.vector.tensor_mul(out=er[:], in0=e1[:], in1=rr[:])
            msk = cp.tile([P, 1], mybir.dt.uint8)  # x >= 0
            nc.vector.tensor_scalar(
                out=msk[:], in0=xs[:], scalar1=0.0, scalar2=None, op0=ALU.is_ge
            )
            p_col = cp.tile([P, 1], F32)
            nc.vector.select(
                out=p_col[:], mask=msk[:], on_true=rr[:], on_false=er[:]
            )
            # 1-p as the reference computes it: exact fp32 subtraction from
            # the already-rounded p (p=1/fp32(1+e) carries the rounding, so
            # 1-p inherits the reference's ladder/flush behavior bit-for-bit)
            omp = cp.tile([P, 1], F32)  # 1 - p
            nc.vector.scalar_tensor_tensor(
                out=omp[:], in0=p_col[:], scalar=-1.0, in1=ones,
                op0=ALU.mult, op1=ALU.add,
            )

            # Z -> 1/Z -> p/Z
            zsum = cp.tile([P, 1], F32)
            nc.vector.tensor_reduce(
                out=zsum[:], in_=zparts[:], axis=mybir.AxisListType.X, op=ALU.add
            )
            rz = cp.tile([P, 1], F32)
            nc.vector.reciprocal(out=rz[:], in_=zsum[:])
            pz = cp.tile([P, 1], F32)
            nc.vector.tensor_mul(out=pz[:], in0=p_col[:], in1=rz[:])

            # everything below the vector pz chain runs post-Z inside the
            # out-stream's DMA slack: the dmat build (reads the deferred aux
            # slice), the C matmul, and the copy-path fix.
            # D[s, k] = 1 iff inv[s] == k, built on-device (f32 exact)
            iota_sb = cp.tile([P, KU], F32)
            nc.gpsimd.iota(
                iota_sb[:], pattern=[[1, KU]], channel_multiplier=0,
                allow_small_or_imprecise_dtypes=True,
            )
            dmat_sb = cp.tile([P, SC, KU], F32)
            for sc in range(SC):
                nc.vector.tensor_scalar(
                    out=dmat_sb[:, sc, :], in0=iota_sb[:],
                    scalar1=aux_sb[:, INV0 + sc : INV0 + sc + 1],
                    scalar2=None, op0=ALU.is_equal,
                )
            # copy-path matmul C = attn @ D
            pC = pp1.tile([P, KU], F32, tag="C")
            for sc in range(SC):
                nc.tensor.matmul(
                    out=pC[:],
                    lhsT=attnT(sc),
                    rhs=dmat_sb[:, sc, :],
                    start=(sc == 0),
                    stop=(sc == SC - 1),
                )

            # pass 2: dense ln(pz*exp + eps) computed in place over the
            # resident exp buffer, streamed straight out. The leading chunks
            # are narrow so the out stream starts as soon after Z as possible
            # (the stream end is DMA-bound, the start gated by the Exp->Ln
            # activation-table switch plus the first Ln). The copy-path
            # fix = ln(pz*exp(lgu) + (1-p)*C + eps) is slotted in mid-stream
            # on the otherwise-idle gpsimd ring once its inputs are safely
            # ready; the out-DMA slack absorbs its scalar time.
            OW = [535, 535, 1070, 1070] + [3210] * 9
            o = 0
            for i, w in enumerate(OW):
                nc.scalar.activation(
                    out=expres[:, o : o + w],
                    in_=expres[:, o : o + w],
                    func=AF.Ln,
                    scale=pz[:, :1],
                    bias=eps_sb[:, :1],
                )
                nc.sync.dma_start(out=outd[:, o : o + w], in_=expres[:, o : o + w])
                o += w
                # copy-path fix slotted mid-stream: the Ln-bearing table set
                # loaded at the Z transition also contains Exp, so neither
                # exp(lgu) nor the fix Ln costs a table switch here. gexp
                # lands after LN5 and the fix Ln after LN6, with the fx
                # vector op computing in between, so the ~1.3us of scalar
                # time is absorbed by the DMA-bound stream's slack and the
                # small fix DMA drains inside the main stream, not after it.
                if i == 5:
                    cs = cp.tile([P, KU], F32)
                    nc.vector.tensor_scalar(
                        out=cs[:], in0=pC[:], scalar1=omp[:, :1],
                        scalar2=None, op0=ALU.mult,
                    )
                    gexp = cp.tile([P, KU], F32)
                    nc.scalar.activation(out=gexp[:], in_=lgu_sb, func=AF.Exp)
                    fx = cp.tile([P, KU], F32)
                    nc.vector.scalar_tensor_tensor(
                        out=fx[:],
                        in0=gexp[:],
                        scalar=pz[:, :1],
                        in1=cs[:],
                        op0=ALU.mult,
                        op1=ALU.add,
                    )
                if i == 6:
                    nc.scalar.activation(
                        out=fx[:], in_=fx[:], func=AF.Ln, bias=eps_sb[:, :1]
                    )
                    nc.gpsimd.dma_start(out=outf[:], in_=fx[:])

    nc.compile()
    return nc


def _host_prep(core: int, logits, attn, enc, dec, wgen, ids):
    """Build one core's pre-laid-out input map (+ its unique-id vector)."""
    b, half = divmod(core, T // P)
    t0 = half * P
    u, inv = np.unique(ids[b], return_inverse=True)

    aux = np.empty((P, AUXW), np.float32)
    # attnT[p, sc*P + t] = attn[b, t0+t, sc*128+p]
    aux[:, AT0:ENC0] = (
        attn[b, t0 : t0 + P].T.reshape(SC, P, P).transpose(1, 0, 2).reshape(P, SC * P)
    )
    # enc[p, sc*H + h] = enc[b, sc*128+p, h]
    aux[:, ENC0:DEC0] = (
        enc[b].reshape(SC, P, H).transpose(1, 0, 2).reshape(P, SC * H)
    )
    # decT[p, c*P + t] = dec[b, t0+t, c*128+p]
    aux[:, DEC0:W0] = (
        dec[b, t0 : t0 + P].T.reshape(HC, P, P).transpose(1, 0, 2).reshape(P, HC * P)
    )
    # wgen[p, c] = W_gen[c*128+p, 0]
    aux[:, W0:LGU0] = wgen.reshape(2 * HC, P).T
    aux[:, LGU0:INV0] = 0.0
    aux[:, LGU0 : LGU0 + len(u)] = logits[b, t0 : t0 + P][:, u]
    # inv[p, sc] = unique-inverse id of source position sc*128+p (exact in f32)
    aux[:, INV0:AUXW] = inv.astype(np.float32).reshape(SC, P).T

    return {
        "lg": np.ascontiguousarray(logits[b, t0 : t0 + P]),
        "aux": np.ascontiguousarray(aux),
    }, u


def kernel(**inputs) -> np.ndarray:
    global LAST_RESULTS
    dec = np.asarray(inputs["decoder_hidden_states"], dtype=np.float32)
    attn = np.asarray(inputs["cross_attention_weights"], dtype=np.float32)
    enc = np.asarray(inputs["encoder_hidden_states"], dtype=np.float32)
    logits = np.asarray(inputs["vocab_logits"], dtype=np.float32)
    wgen = np.asarray(inputs["W_gen"], dtype=np.float32).reshape(2 * H, 1)
    bgen = float(np.asarray(inputs["b_gen"]).reshape(-1)[0])
    ids = np.asarray(inputs["source_ids"]).astype(np.int64)

    key = bgen
    nc = _CACHE.get(key)
    if nc is None:
        nc = _build(bgen)
        _CACHE[key] = nc

    uniqs = []
    in_maps = []
    for core in range(NCORES):
        m, u = _host_prep(core, logits, attn, enc, dec, wgen, ids)
        in_maps.append(m)
        uniqs.append(u)

    trace = bool(os.environ.get("KERNEL_TRACE"))
    if trace:
        _ensure_ntff_hook()
    res = bass_utils.run_bass_kernel_spmd(
        nc,
        in_maps,
        core_ids=list(range(NCORES)),
        trace=trace,
    )
    LAST_RESULTS = res

    out = np.empty((B, T, V), np.float32)
    for core in range(NCORES):
        b, half = divmod(core, T // P)
        t0 = half * P
        r = res.results[core]
        out[b, t0 : t0 + P] = r["outd"]
        u = uniqs[core]
        out[b, t0 : t0 + P, :][:, u] = r["outf"][:, : len(u)]
    return out
